# revision 41
# baseline (speedup 1.0000x reference)
"""GRU-ODE Trainium2 kernel: data-parallel over 8 NeuronCores (16 samples each).

Phases per core:
  1. GRU encoder: 512 sequential steps, hidden state folded as [128, 32]
     (col = half*16 + sample). Per-step critical path shortened: bn folded
     into the Phn matmul group, h-update split as whh@(n*omz) + whh@(zg*h)
     so the next step's matmuls start right after the nz multiply; h itself
     is materialized off-chain on the Pool engine.
  2. Adaptive Dormand-Prince ODE solve: 32 intervals x up-to-16 RK steps with
     exact-identity early exit. Single natural_log_exp table set pinned
     manually (the auto-insertion pass ping-pongs exp/ln sets at ~1.3us per
     reload). Softplus in the symmetric form relu(u) + ln(1+exp(-|u|)) so the
     ln argument stays in (1,2] where the combined table is accurate; the
     relu(u) term is folded into the following matmul (w@l + w@r). FSAL reuse:
     k1 of each step is k7 of the previous accepted step (g-values, stored
     without the dt scale so they survive dt changes).
  3. Readout z @ ro_w.T + ro_b on device, emitted channel-major
     [64, 33*16]; the host transposes to [16, 33, 64] for free.
"""
import sys
import numpy as np

sys.path.insert(0, "/root/.axon_site/_ro/trn_rl_repo")

import concourse.bass as bass
import concourse.bacc as bacc
import concourse.tile as tile
import concourse.mybir as mybir
from contextlib import ExitStack
from concourse.bass import ds
from concourse.bass_utils import run_bass_kernel_spmd

F32 = mybir.dt.float32
AF = mybir.ActivationFunctionType
OP = mybir.AluOpType

B, TIN, NF = 128, 512, 33
CIN, H, COUT, WIDTH = 64, 256, 64, 128
MAX_STEPS = 16
RTOL, ATOL = 1e-3, 1e-6
NCORES = 8
BL = B // NCORES  # 16 samples per core
W2 = 2 * BL

# Dormand-Prince 5(4) tableau
A_TAB = {
    2: [0.2],
    3: [3 / 40, 9 / 40],
    4: [44 / 45, -56 / 15, 32 / 9],
    5: [19372 / 6561, -25360 / 2187, 64448 / 6561, -212 / 729],
    6: [9017 / 3168, -355 / 33, 46732 / 5247, 49 / 176, -5103 / 18656],
}
B5_TAB = {1: 35 / 384, 3: 500 / 1113, 4: 125 / 192, 5: -2187 / 6784, 6: 11 / 84}
E_TAB = {1: 71 / 57600, 3: -71 / 16695, 4: 71 / 1920,
         5: -17253 / 339200, 6: 22 / 525, 7: -1 / 40}
SUM_A = {s: float(sum(A_TAB[s])) for s in A_TAB}
SUM_B5 = float(sum(B5_TAB.values()))
SUM_E = float(sum(E_TAB.values()))
# scaled-identity slots: 0 = I, 1..5 = B5 coeffs (j=1,3,4,5,6), 6..11 = E coeffs
SID_B5 = {j: i + 1 for i, j in enumerate([1, 3, 4, 5, 6])}
SID_E = {j: i + 6 for i, j in enumerate([1, 3, 4, 5, 6, 7])}
NSID = 12


def _prep_weights(inp):
    """Host-side: transform weights into the SBUF layouts the kernel wants."""
    f = lambda a: np.ascontiguousarray(a, dtype=np.float32)
    wih, whh = np.asarray(inp["gru_wih"]), np.asarray(inp["gru_whh"])
    gb, bn = np.asarray(inp["gru_b"]), np.asarray(inp["gru_bn"])
    w0, b0 = np.asarray(inp["w0"]), np.asarray(inp["b0"])
    w1, b1 = np.asarray(inp["w1"]), np.asarray(inp["b1"])
    w2, b2 = np.asarray(inp["w2"]), np.asarray(inp["b2"])
    row, rob = np.asarray(inp["ro_w"]), np.asarray(inp["ro_b"])
    t = np.asarray(inp["t"])

    sid = np.zeros((128, NSID * 128), np.float32)
    eye = np.eye(128, dtype=np.float32)
    sid[:, 0:128] = eye
    for j, slot in SID_B5.items():
        sid[:, slot * 128:(slot + 1) * 128] = eye * np.float32(B5_TAB[j])
    for j, slot in SID_E.items():
        sid[:, slot * 128:(slot + 1) * 128] = eye * np.float32(E_TAB[j])

    w0T = w0.T  # [256, 128]
    w2T = w2.T  # [128, 256]
    roT = row.T  # [256, 64]
    return {
        "wihT": f(np.concatenate([wih.T, gb[None, :]], axis=0)),  # [65, 768]
        "whhT0": f(whh.T[:128]), "whhT1": f(whh.T[128:]),  # [128, 768] each
        "bnr": f(bn[None, :]),  # [1, 256]
        "w0T": f(np.concatenate([w0T[:128], w0T[128:]], axis=1)),  # [128, 256]
        "w1T": f(w1.T),  # [128, 128]
        "w2T": f(w2T),  # [128, 256]
        "b0r": f(b0[None, :]), "b1r": f(b1[None, :]),  # [1, 128]
        "b2r": f(b2[None, :]),  # [1, 256]
        "w0o": f(w0.sum(axis=1)[None, :]),  # [1, 128]
        "roT": f(np.concatenate([roT[:128], roT[128:]], axis=1)),  # [128, 128]
        "rob": f(rob[None, :]),  # [1, 64]
        "sid": f(sid),  # [128, NSID*128]
        "tf": f(t[TIN:][None, :]),  # [1, NF]
    }


def _prep_core_x(y_past, core):
    """y_past [B, TIN, CIN] -> xT_aug [65, TIN*16] for one core, col = t*16+b."""
    yc = np.asarray(y_past, np.float32)[core * BL:(core + 1) * BL]  # [16,T,64]
    xt = yc.transpose(2, 1, 0).reshape(CIN, -1)  # [64, T*16]
    return np.ascontiguousarray(
        np.concatenate([xt, np.ones((1, xt.shape[1]), np.float32)], axis=0))


def _load_act_table(nc, set_name):
    """Manually pin the activation-function table set. The auto-insertion
    pass picks the FIRST set containing each func (exp->exp_and_others,
    ln->natural_log), which ping-pongs ~1.3us reloads inside the ODE loop;
    pre-loading a set that covers every func used downstream makes the
    fixpoint see all paths covered and insert nothing."""
    from concourse.hw_specs import get_activation_tables
    names = list(get_activation_tables(nc.m.arch).keys())
    set_id = names.index(set_name)
    eng = nc.scalar
    return eng.add_instruction(mybir.InstLoadActFuncSet(
        name=eng.bass.get_next_instruction_name(),
        act_func_set_id=set_id, ins=[], outs=[]))


def build_program(tin=TIN, nf=NF, max_steps=MAX_STEPS, check_every=True,
                  fsal=True, debug_dump=False):
    nc = bacc.Bacc("TRN2", target_bir_lowering=False, debug=False)
    d = {}
    d["xT"] = nc.dram_tensor("xT", [CIN + 1, tin * BL], F32, kind="ExternalInput")
    d["tf"] = nc.dram_tensor("tf", [1, nf], F32, kind="ExternalInput")
    for nm, shp in [("wihT", [65, 768]), ("whhT0", [128, 768]), ("whhT1", [128, 768]),
                    ("bnr", [1, 256]), ("w0T", [128, 256]), ("w1T", [128, 128]),
                    ("w2T", [128, 256]), ("b0r", [1, 128]), ("b1r", [1, 128]),
                    ("b2r", [1, 256]), ("w0o", [1, 128]), ("roT", [128, 128]),
                    ("rob", [1, 64]), ("sid", [128, NSID * 128])]:
        d[nm] = nc.dram_tensor(nm, shp, F32, kind="ExternalInput")
    # channel-major device layout [COUT, nf*BL]; host transposes for free
    out_d = nc.dram_tensor("out", [COUT, nf * BL], F32, kind="ExternalOutput")

    ctx = ExitStack()
    tc = ctx.enter_context(tile.TileContext(nc))
    wp = ctx.enter_context(tc.tile_pool(name="w", bufs=1))
    sp = ctx.enter_context(tc.tile_pool(name="s", bufs=1))

    # ---- load weights & inputs ----
    sb = {}
    for nm in ["wihT", "whhT0", "whhT1", "bnr", "w0T", "w1T", "w2T", "b0r",
               "b1r", "b2r", "w0o", "roT", "rob", "sid", "tf"]:
        sb[nm] = wp.tile(list(d[nm].shape), F32, tag=nm, name=nm)
        nc.sync.dma_start(sb[nm][:], d[nm][:])
    xT = wp.tile([CIN + 1, tin * BL], F32, tag="xT")
    nchunk = 4
    cw = tin * BL // nchunk
    for k in range(nchunk):
        nc.sync.dma_start(xT[:, k * cw:(k + 1) * cw], d["xT"][:, k * cw:(k + 1) * cw])

    ones16 = wp.tile([1, BL], F32, tag="ones16")
    onesr = wp.tile([1, 128], F32, tag="onesr")
    onesc = wp.tile([128, 1], F32, tag="onesc")
    eps24 = wp.tile([1, 1], F32, tag="eps24", name="eps24")
    nc.vector.memset(eps24[:], 1e-24)
    nc.vector.memset(ones16[:], 1.0)
    nc.vector.memset(onesr[:], 1.0)
    nc.vector.memset(onesc[:], 1.0)

    # ---- state tiles (fixed addresses; live across dynamic control flow) ----
    z = sp.tile([128, W2], F32, tag="z")          # folded [hidden-half | sample]
    t_st = sp.tile([1, BL], F32, tag="t_st")
    dt_st = sp.tile([1, BL], F32, tag="dt_st")
    zsave = sp.tile([128, nf * W2], F32, tag="zsave")
    ys_sb = sp.tile([COUT, nf * BL], F32, tag="ys")
    g1 = sp.tile([128, W2], F32, tag="g1")        # FSAL: 2*sigmoid(2*v) of f(z)

    # ================= GRU phase =================
    with tc.tile_pool(name="pg", bufs=1, space="PSUM") as pg, \
         tc.tile_pool(name="gs", bufs=1) as gs:
        Gr = pg.tile([128, W2], F32, tag="Gr")
        Gz = pg.tile([128, W2], F32, tag="Gz")
        Phn = pg.tile([128, W2], F32, tag="Phn")
        Pinn = pg.tile([128, W2], F32, tag="Pinn")
        r_sb = gs.tile([128, W2], F32, tag="r_sb")
        z_sb = gs.tile([128, W2], F32, tag="z_sb")
        q30 = gs.tile([128, W2], F32, tag="q30")
        q3 = gs.tile([128, W2], F32, tag="q3")
        n_sb = gs.tile([128, W2], F32, tag="n_sb")
        omz = gs.tile([128, W2], F32, tag="omz")
        zh = gs.tile([128, W2], F32, tag="zh")
        nz = gs.tile([128, W2], F32, tag="nz")
        nc.vector.memset(z[:], 0.0)
        nc.vector.memset(nz[:], 0.0)
        nc.vector.memset(zh[:], 0.0)

        GATES = ((0, Gr), (1, Gz), (2, Phn))

        for t in range(tin):
            xs = xT[:, t * BL:(t + 1) * BL]
            # pass 1: x-only parts (run while the previous step's elementwise
            # is still in flight) — wih for all gates, bn row for n, inn
            # PSUM start-flag semantics: start lazily invalidates the WHOLE
            # 2KB bank, so each gate tile gets exactly ONE group per step —
            # start=True only on the first matmul touching the tile; bytes
            # still marked pending are overwritten (not accumulated) on
            # first touch, which is exactly the per-half fresh-write we want.
            for gate, P in GATES:
                for half in (0, 1):
                    col = gate * 256 + half * 128
                    o = P[:, half * BL:(half + 1) * BL]
                    if gate == 2:
                        # Phn holds only whh@h + bn (the r-gated part); the
                        # x-part (inn) lives in Pinn
                        nc.tensor.matmul(o, sb["bnr"][0:1, half * 128:half * 128 + 128],
                                         ones16[:], start=(half == 0), stop=False,
                                         skip_group_check=True)
                    else:
                        nc.tensor.matmul(o, sb["wihT"][:, col:col + 128],
                                         xs, start=(half == 0), stop=False,
                                         skip_group_check=True)
            for half in (0, 1):  # inn: wih only
                col = 2 * 256 + half * 128
                oi = Pinn[:, half * BL:(half + 1) * BL]
                nc.tensor.matmul(oi, sb["wihT"][:, col:col + 128], xs,
                                 start=(half == 0), stop=(half == 1),
                                 skip_group_check=True)
            # pass 2: whh @ zh (zg*h from the previous step, ready early)
            for gate, P in GATES:
                for half in (0, 1):
                    col = gate * 256 + half * 128
                    o = P[:, half * BL:(half + 1) * BL]
                    nc.tensor.matmul(o, sb["whhT0"][:, col:col + 128],
                                     zh[:, 0:BL], start=False, stop=False,
                                     skip_group_check=True)
                    nc.tensor.matmul(o, sb["whhT1"][:, col:col + 128],
                                     zh[:, BL:W2], start=False, stop=False,
                                     skip_group_check=True)
            # pass 3: whh @ nz (tanh-gated update, the critical-path input);
            # Gr first so sigmoid(r) starts as early as possible
            for gate, P in GATES:
                for half in (0, 1):
                    col = gate * 256 + half * 128
                    o = P[:, half * BL:(half + 1) * BL]
                    nc.tensor.matmul(o, sb["whhT0"][:, col:col + 128],
                                     nz[:, 0:BL], start=False, stop=False,
                                     skip_group_check=True)
                    nc.tensor.matmul(o, sb["whhT1"][:, col:col + 128],
                                     nz[:, BL:W2], start=False, stop=(half == 1),
                                     skip_group_check=True)

            nc.scalar.activation(r_sb[:], Gr[:], AF.Sigmoid)
            nc.scalar.activation(z_sb[:], Gz[:], AF.Sigmoid)
            nc.vector.tensor_tensor(q30[:], r_sb[:], Phn[:], OP.mult)
            nc.vector.tensor_tensor(q3[:], q30[:], Pinn[:], OP.add)
            nc.scalar.activation(n_sb[:], q3[:], AF.Tanh)
            nc.gpsimd.tensor_scalar(omz[:], z_sb[:], -1.0, 1.0, OP.mult, OP.add)
            nc.gpsimd.tensor_tensor(zh[:], z_sb[:], z[:], OP.mult)
            nc.vector.tensor_tensor(nz[:], n_sb[:], omz[:], OP.mult)
            nc.gpsimd.tensor_tensor(z[:], nz[:], zh[:], OP.add)

    nc.vector.tensor_copy(zsave[:, 0:W2], z[:])

    # ================= ODE phase =================
    # All activations below use only Exp/Ln -> single table set.
    with tc.tile_pool(name="po", bufs=1, space="PSUM") as po, \
         tc.tile_pool(name="os", bufs=1) as osb:
        P0 = po.tile([128, W2], F32, tag="P0")   # dt-broadcast / B5-sum / accept
        P1 = po.tile([128, W2], F32, tag="P1")   # odt corrections / err sum
        P2 = po.tile([128, BL], F32, tag="P2")   # u (w0@z + b0) / u7
        P3 = po.tile([128, BL], F32, tag="P3")   # layer-1 preact
        P4 = po.tile([128, W2], F32, tag="P4")   # layer-2 preact
        P5 = po.tile([128, BL], F32, tag="P5")   # w0@g_j / msq reduce
        scr = po.tile([128, 8 * BL], F32, tag="scr")  # act/DVE scratch bank
        ap = scr[:, 0 * BL:1 * BL]    # |u|
        ep = scr[:, 1 * BL:2 * BL]    # exp(-|u|)
        a1p = scr[:, 2 * BL:3 * BL]
        e1p = scr[:, 3 * BL:4 * BL]
        etp = scr[:, 4 * BL:6 * BL]
        ddp = scr[:, 6 * BL:8 * BL]

        l0 = osb.tile([128, BL], F32, tag="l0")
        r0 = osb.tile([128, BL], F32, tag="r0")
        l1 = osb.tile([128, BL], F32, tag="l1")
        r1 = osb.tile([128, BL], F32, tag="r1")
        gg = {j: osb.tile([128, W2], F32, tag=f"gg{j}", name=f"gg{j}")
              for j in range(2, 8)}
        acc = {s: osb.tile([128, BL], F32, tag=f"acc{s}", name=f"acc{s}")
               for s in range(2, 7)}
        P5d = osb.tile([128, BL], F32, tag="P5d")
        ub = osb.tile([128, BL], F32, tag="ub")
        dtb2 = osb.tile([128, W2], F32, tag="dtb2")
        y5sb = osb.tile([128, W2], F32, tag="y5sb")
        y5t = osb.tile([128, W2], F32, tag="y5t")
        scm = osb.tile([128, W2], F32, tag="scm")
        qt = osb.tile([128, W2], F32, tag="qt")
        q2 = osb.tile([128, W2], F32, tag="q2")
        L16 = lambda tg: osb.tile([1, BL], F32, tag=tg, name=tg)
        rem, mx, dt_use = L16("rem"), L16("mx"), L16("dt_use")
        nd, done = L16("nd"), L16("done")
        dt2 = osb.tile([1, W2], F32, tag="dt2")
        tm, lnm, f0 = L16("tm"), L16("lnm"), L16("f0")
        rcd = osb.tile([128, W2], F32, tag="rcd", name="rcd")
        rsc = osb.tile([128, W2], F32, tag="rsc", name="rsc")
        rscd = osb.tile([128, W2], F32, tag="rscd", name="rscd")
        mx1 = osb.tile([128, W2], F32, tag="mx1", name="mx1")
        mx2 = osb.tile([128, W2], F32, tag="mx2", name="mx2")
        accI = osb.tile([128, W2], mybir.dt.int32, tag="accI", name="accI")
        doneI = osb.tile([1, BL], mybir.dt.int32, tag="doneI", name="doneI")
        fac, fac2, le, acc16 = L16("fac"), L16("fac2"), L16("le"), L16("acc16")
        acc32 = osb.tile([1, W2], F32, tag="acc32")
        st_t, cand, cand2 = L16("st_t"), L16("cand"), L16("cand2")
        remn, ndn = L16("remn"), L16("ndn")
        flag = osb.tile([1, 1], F32, tag="flag")
        flagi = [osb.tile([1, 1], mybir.dt.int32, tag=f"flagi{s}", name=f"flagi{s}")
                 for s in range(MAX_STEPS)]
        tmpF, tmpL = L16("tmpF"), L16("tmpL")

        tf_sb = sb["tf"]
        # dt0 = (tf[-1] - tf[0]) * 0.01
        nc.vector.tensor_scalar(tmpF[:], ones16[:], tf_sb[0:1, 0:1], None, OP.mult)
        nc.vector.scalar_tensor_tensor(tmpL[:], ones16[:], tf_sb[0:1, nf - 1:nf],
                                       tmpF[:], OP.mult, OP.subtract)
        nc.vector.tensor_scalar(dt_st[:], tmpL[:], 0.01, None, OP.mult)
        # exp+ln table loads are spliced in post-compile (_fixup_act_tables);
        # loads emitted here have no data deps and get hoisted by the
        # scheduler to the stream head, leaving the wrong table resident

        def w0mm(dst, rhs32, extra_b0=False):
            nc.tensor.matmul(dst, sb["w0T"][:, 0:128], rhs32[:, 0:BL],
                             start=True, stop=False)
            nc.tensor.matmul(dst, sb["w0T"][:, 128:256], rhs32[:, BL:W2],
                             start=False, stop=not extra_b0)
            if extra_b0:
                nc.tensor.matmul(dst, sb["b0r"][:], ones16[:],
                                 start=False, stop=True)

        def emit_mlp(u_src, g_out, u_in_sbuf=False):
            """u_src: [128, BL] preactivation AP (b0 included; PSUM or SBUF).
            g_out: [128, W2] SBUF result = 2*sigmoid(2*v) = tanh(v)+1."""
            # softplus layer 0, symmetric form relu(u) + ln(1+exp(-|u|));
            # |u| on the Act engine (abs_max isn't a legal DVE tensor_scalar);
            # relu on Pool when the source is SBUF, keeping the DVE queue
            # clear for the P5d/acc chains (Pool can't read PSUM)
            nc.scalar.activation(ap, u_src, AF.Abs)
            reng = nc.gpsimd if u_in_sbuf else nc.vector
            reng.tensor_scalar(r0[:], u_src, 0.0, None, OP.max)
            nc.scalar.activation(ep, ap, AF.Exp, scale=-1.0)
            nc.scalar.activation(l0[:], ep, AF.Ln, bias=1.0)
            nc.tensor.matmul(P3[:], sb["w1T"][:], l0[:], start=True, stop=False)
            nc.tensor.matmul(P3[:], sb["w1T"][:], r0[:], start=False, stop=False)
            nc.tensor.matmul(P3[:], sb["b1r"][:], ones16[:], start=False, stop=True)
            # softplus layer 1
            nc.scalar.activation(a1p, P3[:], AF.Abs)
            nc.vector.tensor_scalar(r1[:], P3[:], 0.0, None, OP.max)
            nc.scalar.activation(e1p, a1p, AF.Exp, scale=-1.0)
            nc.scalar.activation(l1[:], e1p, AF.Ln, bias=1.0)
            # head: v = w2@(l1+r1) + b2 ; g = 2*sigmoid(2v) = 1/(.5+.5e^-2v)
            for half in (0, 1):
                o = P4[:, half * BL:(half + 1) * BL]
                cols = slice(half * 128, (half + 1) * 128)
                nc.tensor.matmul(o, sb["w2T"][:, cols], l1[:], start=True, stop=False)
                nc.tensor.matmul(o, sb["w2T"][:, cols], r1[:], start=False, stop=False)
                nc.tensor.matmul(o, sb["b2r"][0:1, cols], ones16[:],
                                 start=False, stop=True)
            nc.scalar.activation(etp, P4[:], AF.Exp, scale=-2.0)
            nc.vector.tensor_scalar(ddp, etp, 0.5, 0.5, OP.mult, OP.add)
            nc.vector.reciprocal_approx_fast(out=g_out, in_=ddp)

        def g_feed(j, g_t):
            """After g_j lands: accumulate B5/E sums and the acc[] updates."""
            if j in SID_B5:
                slot = SID_B5[j]
                nc.tensor.matmul(P0[:], sb["sid"][:, slot * 128:(slot + 1) * 128],
                                 g_t[:], start=(j == 1), stop=(j == 6))
            if j in SID_E:
                slot = SID_E[j]
                nc.tensor.matmul(P1[:], sb["sid"][:, slot * 128:(slot + 1) * 128],
                                 g_t[:], start=(j == 1), stop=(j == 7))
            if j <= 5:
                w0mm(P5[:], g_t)
                nc.vector.tensor_tensor(P5d[:], P5[:], dtb2[:, 0:BL], OP.mult)
                for s2 in range(max(j + 1, 2), 7):
                    nc.vector.scalar_tensor_tensor(
                        acc[s2][:], P5d[:], float(A_TAB[s2][j - 1]), acc[s2][:],
                        OP.mult, OP.add)

        # FSAL init: g1 = 2*sigmoid(2*f-head(zc)) evaluated once
        w0mm(P2[:], z, extra_b0=True)
        emit_mlp(P2[:], g1[:])

        def emit_step(tnext_ap, sidx):
            # lane control at step start
            nc.vector.tensor_scalar(rem[:], t_st[:], -1.0, tnext_ap, OP.mult, OP.add)
            # dt_use = min(dt, max(rem, 0)) fused into one DVE op — this is
            # the head of the step's critical chain (gates the dt broadcast)
            nc.vector.scalar_tensor_tensor(dt_use[:], rem[:], 0.0, dt_st[:],
                                           OP.max, OP.min)
            nc.vector.tensor_copy(dt2[0:1, 0:BL], dt_use[:])
            nc.vector.tensor_copy(dt2[0:1, BL:W2], dt_use[:])
            nc.tensor.matmul(P0[:], onesr[:], dt2[:], start=True, stop=True)
            nc.vector.tensor_copy(dtb2[:], P0[:])
            # nd/done are consumed only by the late accept/dt blends; keep
            # them off the step-head DVE queue that gates the dt broadcast
            nc.vector.tensor_scalar(nd[:], rem[:], 1e-8, None, OP.is_gt)
            nc.vector.tensor_scalar(done[:], rem[:], 1e-8, None, OP.is_le)
            nc.tensor.matmul(P1[:, 0:BL], sb["w0o"][:], dt_use[:],
                             start=True, stop=True)
            w0mm(P2[:], z, extra_b0=True)
            # PSUM port rule: only one non-scalar PSUM input per DVE op,
            # so stage ub = w0@z + b0 through SBUF before the acc inits
            nc.vector.tensor_copy(ub[:], P2[:])
            for s in range(2, 7):
                nc.vector.scalar_tensor_tensor(acc[s][:], P1[:, 0:BL], -SUM_A[s],
                                               ub[:], OP.mult, OP.add)
            if not fsal:  # diagnostic: fresh stage-1 eval every step
                emit_mlp(ub[:], g1[:])
            g_feed(1, g1)
            for s in range(2, 7):
                emit_mlp(acc[s][:], gg[s][:], u_in_sbuf=True)
                g_feed(s, gg[s])
            # y5 = z + dt*(B5sum - SUM_B5)
            nc.vector.scalar_tensor_tensor(y5t[:], P0[:], -SUM_B5, dtb2[:],
                                           OP.add, OP.mult)
            nc.vector.tensor_tensor(y5sb[:], y5t[:], z[:], OP.add)
            # k7 = f(y5); meanwhile error-scale factors (off critical path)
            w0mm(P2[:], y5sb, extra_b0=True)
            nc.vector.tensor_tensor(mx1[:], z[:], y5sb[:], OP.max)
            nc.vector.tensor_tensor(mx2[:], z[:], y5sb[:], OP.min)
            nc.vector.scalar_tensor_tensor(scm[:], mx2[:], -1.0, mx1[:],
                                           OP.mult, OP.max)
            nc.gpsimd.tensor_scalar(scm[:], scm[:], RTOL, ATOL, OP.mult, OP.add)
            nc.vector.reciprocal_approx_fast(out=rsc[:], in_=scm[:])
            nc.vector.tensor_tensor(rscd[:], rsc[:], dtb2[:], OP.mult)
            emit_mlp(P2[:], gg[7][:])
            g_feed(7, gg[7])
            # qt = err/scale = (Esum - SUM_E)*dt*rsc ; msq via ones-reduce
            nc.vector.scalar_tensor_tensor(qt[:], P1[:], -SUM_E, rscd[:],
                                           OP.add, OP.mult)
            nc.vector.tensor_tensor(q2[:], qt[:], qt[:], OP.mult)
            # both halves accumulate into one PSUM region: tm = sum over 256
            nc.tensor.matmul(P5[0:1, 0:BL], onesc[:], q2[:, 0:BL],
                             start=True, stop=False)
            nc.tensor.matmul(P5[0:1, 0:BL], onesc[:], q2[:, BL:W2],
                             start=False, stop=True)
            tm_ap = P5[0:1, 0:BL]
            # factor = clip(0.9 * msq^-0.1, 0.2, 10); msq = tm/256
            nc.scalar.activation(lnm[:], tm_ap, AF.Ln, scale=1.0 / 256.0,
                                 bias=eps24[0:1, 0:1])
            nc.scalar.activation(f0[:], lnm[:], AF.Exp, scale=-0.1)
            nc.vector.tensor_scalar(fac[:], f0[:], 0.9, 0.2, OP.mult, OP.max)
            nc.vector.tensor_scalar(fac2[:], fac[:], 10.0, None, OP.min)
            # accept = (msq <= 1) & notdone   (tm <= 256)
            nc.vector.tensor_scalar(le[:], tm_ap, 256.0, None, OP.is_le)
            nc.vector.tensor_tensor(acc16[:], le[:], nd[:], OP.mult)
            # finalize the next step's gating state first (t, flag, dt) so
            # the sequencers' flag load and the branch overlap the accept
            # blends below instead of serializing after them
            nc.vector.tensor_tensor(st_t[:], acc16[:], dt_use[:], OP.mult)
            nc.vector.tensor_tensor(t_st[:], t_st[:], st_t[:], OP.add)
            nc.vector.tensor_scalar(remn[:], t_st[:], -1.0, tnext_ap,
                                    OP.mult, OP.add)
            nc.vector.tensor_scalar(ndn[:], remn[:], 1e-8, None, OP.is_gt)
            nc.vector.reduce_max(flag[:], ndn[:], axis=mybir.AxisListType.X)
            nc.vector.tensor_copy(flagi[sidx][:], flag[:])
            nc.vector.tensor_copy(flagi[sidx][:], flag[:])
            # dt' = done ? dt : max(dt_use*factor, 1e-6). For done lanes
            # dt_use=0 so cand=0 and dt>=1e-6 always, hence the max-blend
            # dt' = max(cand, dt*done, 1e-6) is exact and branch-free.
            nc.vector.tensor_tensor(cand[:], dt_use[:], fac2[:], OP.mult)
            nc.vector.tensor_tensor(cand2[:], dt_st[:], done[:], OP.mult)
            nc.vector.scalar_tensor_tensor(dt_st[:], cand[:], 1e-6, cand2[:],
                                           OP.max, OP.max)
            # accept blends (consumed later, at the next step's w0mm)
            nc.vector.tensor_copy(acc32[0:1, 0:BL], acc16[:])
            nc.vector.tensor_copy(acc32[0:1, BL:W2], acc16[:])
            nc.tensor.matmul(P0[:], onesr[:], acc32[:], start=True, stop=True)
            nc.vector.tensor_copy(accI[:], P0[:])
            nc.vector.copy_predicated(z[:], accI[:], y5sb[:])
            nc.vector.copy_predicated(g1[:], accI[:], gg[7][:])

        with tc.For_i(1, nf) as iv:
            tprev_ap = tf_sb[0:1, ds(iv - 1, 1)]
            tnext_ap = tf_sb[0:1, ds(iv, 1)]
            nc.vector.tensor_scalar(t_st[:], ones16[:], tprev_ap, None, OP.mult)
            emit_step(tnext_ap, 0)
            if check_every:
                with ExitStack() as stk:
                    for s in range(1, max_steps):
                        v = nc.values_load(flagi[s - 1][0:1, 0:1],
                                           skip_runtime_bounds_check=True)
                        stk.enter_context(tc.If(v > 0))
                        emit_step(tnext_ap, s)
            else:
                # diagnostic: unconditional steps (done lanes are exact no-ops)
                for s in range(1, max_steps):
                    emit_step(tnext_ap, s)
            off = nc.snap(iv * W2)
            nc.vector.tensor_copy(zsave[:, ds(off, W2)], z[:])

        if debug_dump:
            dbg = {"ub": ub, "dtb2": dtb2, "y5t": y5t, "y5sb": y5sb,
                   "rscd": rscd, "qt": qt, "q2": q2, "acc16": acc16,
                   "le": le, "fac2": fac2, "dt_st": dt_st, "t_st": t_st,
                   "zf": z, "dt_use": dt_use, "g1d": g1}
            for s in range(2, 7):
                dbg[f"accd{s}"] = acc[s]
            for j in range(2, 8):
                dbg[f"ggd{j}"] = gg[j]
            for nm, t_ in dbg.items():
                dd_ = nc.dram_tensor(f"dbg_{nm}", list(t_.shape), F32,
                                     kind="ExternalOutput")
                nc.sync.dma_start(dd_[:], t_[:])

    # ================= readout =================
    with tc.tile_pool(name="pr", bufs=2, space="PSUM") as pr:
        for s in range(nf):
            rop = pr.tile([COUT, BL], F32, tag="rop")
            nc.tensor.matmul(rop[:], sb["roT"][:, 0:COUT], zsave[:, s * W2:s * W2 + BL],
                             start=True, stop=False)
            nc.tensor.matmul(rop[:], sb["roT"][:, COUT:2 * COUT],
                             zsave[:, s * W2 + BL:(s + 1) * W2], start=False, stop=False)
            nc.tensor.matmul(rop[:], sb["rob"][:], ones16[:], start=False, stop=True)
            nc.scalar.copy(ys_sb[:, s * BL:(s + 1) * BL], rop[:])
    nc.sync.dma_start(out_d[:], ys_sb[:])

    ctx.close()
    return nc


_CACHE = {}


def _strip_redundant_act_loads(nc):
    """Post-compile activation-table fixup.

    The auto-insertion pass pairs each Exp with exp_and_others and each Ln
    with natural_log (first set containing the func), ping-ponging ~1.3us
    table reloads inside the ODE loop. natural_log_exp_and_others covers
    both, so: (1) strip every exp_and_others/natural_log load, then
    (2) splice one natural_log_exp_and_others load per block that executes
    Exp/Ln — positioned after the GRU's last Sigmoid/Tanh in the main block,
    at the top of the Act stream elsewhere. This runs after semaphore
    generation (like the insertion pass itself), so loads carry no sync
    edges and placement is by final stream position, which is what both the
    runtime and walrus's per-instruction function encoding key off.
    """
    from concourse.hw_specs import get_activation_tables
    names = list(get_activation_tables(nc.m.arch).keys())
    drop = {names.index("exp_and_others"), names.index("natural_log")}
    nlexp = names.index("natural_log_exp_and_others")
    gru_f = {AF.Sigmoid, AF.Tanh}
    tbl_f = {AF.Exp, AF.Ln}
    removed = 0
    for b in nc.m.functions[0].blocks:
        keep = []
        for i in b.instructions:
            if (isinstance(i, mybir.InstLoadActFuncSet)
                    and i.act_func_set_id in drop):
                si = i.sync_info
                assert si is None or (len(si.on_wait) == 0 and
                                      len(si.on_update) == 0), \
                    f"act-table load {i.name} has sync edges; cannot strip"
                removed += 1
                continue
            keep.append(i)
        b.instructions[:] = keep

        has_tbl = any(isinstance(i, mybir.InstActivation) and i.func in tbl_f
                      for i in b.instructions)
        has_gru = any(isinstance(i, mybir.InstActivation) and i.func in gru_f
                      for i in b.instructions)
        # Only the block that transitions away from the sigmoid set needs a
        # spliced load; pure-Exp/Ln blocks (the ODE loop body and If-step
        # blocks) inherit the set from their predecessors, skipping a
        # 1.28us reload per loop iteration.
        if not has_tbl or not has_gru:
            continue
        last_gru = -1
        for idx, i in enumerate(b.instructions):
            if isinstance(i, mybir.InstActivation) and i.func in gru_f:
                last_gru = idx
        ins_at = None
        for idx, i in enumerate(b.instructions):
            if idx <= last_gru:
                continue
            if (i.engine == mybir.EngineType.Activation
                    and isinstance(i, mybir.InstActivation)):
                ins_at = idx
                break
        if ins_at is None:
            continue
        load = mybir.InstLoadActFuncSet(
            name=nc.get_next_instruction_name(),
            act_func_set_id=nlexp, ins=[], outs=[])
        load.engine = mybir.EngineType.Activation
        nc.register_instruction(load)
        b.instructions.insert(ins_at, load)
    return removed


def _get_program():
    if "nc" not in _CACHE:
        nc = build_program()
        nc.compile()
        _strip_redundant_act_loads(nc)
        _CACHE["nc"] = nc
    return _CACHE["nc"]


def kernel(**inputs):
    nc = _get_program()
    w = _prep_weights(inputs)
    in_maps = []
    for c in range(NCORES):
        m = dict(w)
        m["xT"] = _prep_core_x(inputs["y_past"], c)
        in_maps.append(m)
    res = run_bass_kernel_spmd(nc, in_maps, list(range(NCORES)))
    out = np.stack([np.asarray(res.results[c]["out"])
                    .reshape(COUT, NF, BL).transpose(2, 1, 0)
                    for c in range(NCORES)])
    return np.ascontiguousarray(out.reshape(B, NF, COUT))
